# revision 16
# baseline (speedup 1.0000x reference)
"""RelPatchAttention2D kernel for Trainium2 (Bass/Tile), data-parallel over
batch across 8 NeuronCores.

Per sample (one core): q,k,v [256,128,128] f32. Patches: (s0, i, j) with
s0 = C//64 (4), i = H//16 (8), j = W//16 (8) -> 256 patches of 64*16*16 =
16384 elements. Computes the all-pairs patch Gram matrix G = Qp @ Kp^T,
sim = (G+eps)/(qq[:,None]+kk[None,:]-G+eps), t = w @ sim + b, and
out = t[patch(v)] * v.

Pipeline per core:
  - q,k streamed in natural layout [C-half, hh-half band, W] tiles,
    reordered on DVE to (j, hh, ww)-blocked float32r layout, PE-transposed
    into patch-major W buffers (free = cc*256 + patch), phased over
    hh-halves so both W buffers fit in SBUF.
  - G (and the q/k self-Grams, for their diagonals qq/kk) accumulated in
    PSUM over 2 phases x 64 cc-chunks x 2 M-halves of [K=128, M=128,
    N=256] float32r matmuls.
  - diag extraction via identity-mask + row reduce; row<->column and
    row->all-partition broadcasts via tiny K=1 matmuls / PE transposes.
  - v scaled per patch with broadcast-AP DVE multiplies.
"""

import numpy as np

import concourse.bass as bass
import concourse.tile as tile
from concourse import mybir
from concourse.bass_utils import run_bass_kernel_spmd

F32 = mybir.dt.float32
F32R = mybir.dt.float32r
N_CORES = 8
C, H, W = 256, 128, 128
SMOOTH = 1e-05


def split_excess_waits(nc, max_waits=1):
    # This walrus build rejects >1 sync-wait per instruction ("Too many
    # sync wait commands"); move extras onto same-engine NOPs inserted
    # directly before the instruction.
    ctr = 0
    for f in nc.m.functions:
        for b in f.blocks:
            new_list = []
            changed = False
            for inst in b.instructions:
                si = getattr(inst, "sync_info", None)
                if si is not None and si.on_wait and len(si.on_wait) > max_waits:
                    waits = list(si.on_wait)
                    for w in waits[:-max_waits]:
                        nop = mybir.InstNoOp(name=f"wsplit-{ctr}", ins=[], outs=[])
                        ctr += 1
                        nop.engine = inst.engine
                        nop.sync_info = mybir.SyncInfo(on_wait=[w], on_update=[])
                        new_list.append(nop)
                    si.on_wait = waits[-max_waits:]
                    changed = True
                new_list.append(inst)
            if changed:
                b.instructions = new_list


def build_kernel(debug=False):
    nc = bass.Bass("TRN2", target_bir_lowering=False, debug=False)

    q = nc.dram_tensor("q", [C, H, W], F32, kind="ExternalInput").ap()
    k = nc.dram_tensor("k", [C, H, W], F32, kind="ExternalInput").ap()
    v = nc.dram_tensor("v", [C, H, W], F32, kind="ExternalInput").ap()
    w_shrink = nc.dram_tensor("w_shrink", [1, 256], F32, kind="ExternalInput").ap()
    b_shrink = nc.dram_tensor("b_shrink", [1, 1], F32, kind="ExternalInput").ap()
    ident = nc.dram_tensor("ident", [128, 128], F32, kind="ExternalInput").ap()
    ones_r = nc.dram_tensor("ones_r", [1, 128], F32, kind="ExternalInput").ap()
    out = nc.dram_tensor("out", [C, H, W], F32, kind="ExternalOutput").ap()
    if debug:
        dbg_G = nc.dram_tensor("dbg_G", [128, 512], F32, kind="ExternalOutput").ap()
        dbg_qq = nc.dram_tensor("dbg_qq", [128, 2], F32, kind="ExternalOutput").ap()
        dbg_kk = nc.dram_tensor("dbg_kk", [128, 2], F32, kind="ExternalOutput").ap()
        dbg_t = nc.dram_tensor("dbg_t", [1, 256], F32, kind="ExternalOutput").ap()

    qk_dram = (q, k)

    with tile.TileContext(nc) as tc:
        with (
            tc.tile_pool(name="aux", bufs=1) as aux_pool,
            tc.tile_pool(name="wbuf", bufs=1) as w_pool,
            tc.tile_pool(name="stage1", bufs=3) as s1_pool,
            tc.tile_pool(name="stage2", bufs=4) as s2_pool,
            tc.tile_pool(name="small", bufs=1) as small_pool,
            tc.tile_pool(name="vio", bufs=2) as v_pool,
            tc.tile_pool(name="voo", bufs=2) as o_pool,
            tc.tile_pool(name="tps", bufs=2, space="PSUM") as tp_psum,
            tc.tile_pool(name="gps", bufs=1, space="PSUM") as g_psum,
            tc.tile_pool(name="mps", bufs=1, space="PSUM") as m_psum,
        ):
            ident_sb = aux_pool.tile([128, 128], F32)
            nc.sync.dma_start(ident_sb[:], ident[:, :])
            identr_sb = aux_pool.tile([128, 128], F32R)
            nc.vector.tensor_copy(identr_sb[:], ident_sb[:])
            ones_sb = aux_pool.tile([1, 128], F32)
            nc.sync.dma_start(ones_sb[:], ones_r[:, :])
            w_sb = aux_pool.tile([1, 256], F32)
            nc.sync.dma_start(w_sb[:], w_shrink[:, :])
            b_sb = aux_pool.tile([1, 1], F32)
            nc.sync.dma_start(b_sb[:], b_shrink[:, :])

            # persistent merged W buffer: [128 hw, cc*512 + tk*256 + p]
            # float32r (tk: 0 = k, 1 = q), one hh-half phase at a time
            W_all = w_pool.tile([128, 64 * 512], F32R, name="W_all", tag="W_all")
            # Gram accumulators: GGqq[mh] = [G-row (vs k, 256) | Gqq-row
            # (vs q, 256)]; Gkk regions by M-half
            GGqq_ps = [
                g_psum.tile([128, 512], F32, name=f"GGqq{m}", tag=f"GGqq{m}")
                for m in range(2)
            ]
            Gkk_ps = g_psum.tile([128, 512], F32, name="Gkk_ps", tag="Gkk_ps")

            for h0 in range(2):
                for t in range(2):
                    src = qk_dram[t]
                    for ch in range(2):
                        for i in range(8):
                            st1 = s1_pool.tile([128, 1024], F32)
                            nc.sync.dma_start(
                                st1[:].rearrange("p (hh w) -> p hh w", hh=8),
                                src[
                                    ch * 128:(ch + 1) * 128,
                                    i * 16 + h0 * 8: i * 16 + h0 * 8 + 8,
                                    :,
                                ],
                            )
                            # reorder (hh, j, ww) -> (j, hh, ww), cast f32r
                            st2 = s2_pool.tile([128, 1024], F32R)
                            nc.vector.tensor_copy(
                                st2[:].rearrange("p (j hh w) -> p j hh w", j=8, hh=8),
                                st1[:].rearrange("p (hh j w) -> p j hh w", hh=8, j=8),
                            )
                            # transposes into W; 8 j-blocks -> one 2-bank
                            # PSUM group, one scattered copy (split ACT/DVE)
                            wv = W_all[:].rearrange(
                                "p (cc tk s i j) -> p tk j s cc i",
                                cc=64, tk=2, s=4, i=8, j=8,
                            )[:, 1 - t]
                            ps = tp_psum.tile([128, 1024], F32R)
                            for j in range(8):
                                nc.tensor.transpose(
                                    ps[:, j * 128:(j + 1) * 128],
                                    st2[:, j * 128:(j + 1) * 128],
                                    identr_sb[:],
                                )
                            wdst = wv[:, :, 2 * ch:2 * ch + 2, :, i]
                            wsrc = ps[:].rearrange(
                                "p (j s cc) -> p j s cc", j=8, s=2
                            )
                            if i % 3 < 2:
                                nc.scalar.copy(wdst, wsrc)
                            else:
                                nc.vector.tensor_copy(wdst, wsrc)
                # Gram chunks for this phase: [G | Gqq] = Wq^T [Wk | Wq]
                # in one N=512 matmul per M-half; Gkk = Wk^T Wk (N=256)
                lw = W_all[:].rearrange("p (cc pp) -> p cc pp", cc=64)
                first = h0 == 0
                last = h0 == 1
                for cc_i in range(64):
                    # start=True clears the WHOLE PSUM bank, so only the
                    # very first matmul into each bank may carry it; other
                    # groups in the bank begin with start=False (their
                    # region's has_written is clear, so the first write
                    # lands in overwrite mode).
                    st = (first and cc_i == 0)
                    sp = (last and cc_i == 63)
                    for mh in range(2):
                        lhs_q = lw[:, cc_i, 256 + mh * 128: 256 + (mh + 1) * 128]
                        lhs_k = lw[:, cc_i, mh * 128:(mh + 1) * 128]
                        nc.tensor.matmul(
                            GGqq_ps[mh][:],
                            lhs_q, lw[:, cc_i, :],
                            start=st, stop=sp,
                            skip_group_check=True,
                        )
                        nc.tensor.matmul(
                            Gkk_ps[:, mh * 256:(mh + 1) * 256],
                            lhs_k, lw[:, cc_i, 0:256],
                            start=(st and mh == 0), stop=sp,
                            skip_group_check=True,
                        )

            # ---- qq/kk columns: diag(selfgram) via identity mask + reduce
            qq_col = [
                small_pool.tile([128, 1], F32, name=f"qqc{m}", tag=f"qqc{m}")
                for m in range(2)
            ]
            kk_col = [
                small_pool.tile([128, 1], F32, name=f"kkc{m}", tag=f"kkc{m}")
                for m in range(2)
            ]
            w_col = [
                small_pool.tile([128, 1], F32, name=f"wc{m}", tag=f"wc{m}")
                for m in range(2)
            ]
            dtmp = small_pool.tile([128, 128], F32, name="dtmp", tag="dtmp")
            for mh in range(2):
                for gslice, col in (
                    (GGqq_ps[mh][:, 256 + mh * 128: 256 + (mh + 1) * 128],
                     qq_col[mh]),
                    (Gkk_ps[:, mh * 256 + mh * 128: mh * 256 + (mh + 1) * 128],
                     kk_col[mh]),
                ):
                    nc.vector.tensor_mul(dtmp[:], gslice, ident_sb[:])
                    nc.vector.tensor_reduce(
                        col[:], dtmp[:],
                        axis=mybir.AxisListType.X, op=mybir.AluOpType.add,
                    )
            # eps folded into qq
            for mh in range(2):
                nc.vector.tensor_scalar_add(qq_col[mh][:], qq_col[mh][:], SMOOTH)

            # kk as a broadcast row: transpose columns -> row, then K=1 bcast
            kr_ps = m_psum.tile([1, 256], F32, tag="misc", name="kr_ps")
            for mh in range(2):
                nc.tensor.transpose(
                    kr_ps[0:1, mh * 128:(mh + 1) * 128], kk_col[mh][:, 0:1],
                    ident_sb[:],
                )
            kk_row = small_pool.tile([1, 256], F32)
            nc.vector.tensor_copy(kk_row[:], kr_ps[:])
            kkb_ps = m_psum.tile([128, 256], F32, tag="misc", name="kkb_ps")
            nc.tensor.matmul(kkb_ps[:], ones_sb[0:1, :], kk_row[0:1, :],
                             start=True, stop=True, skip_group_check=True)
            kk_bc = small_pool.tile([128, 256], F32)
            nc.vector.tensor_copy(kk_bc[:], kkb_ps[:])

            # w as per-partition columns
            wc_ps = m_psum.tile([128, 64], F32, tag="misc", name="wc_ps")
            for mh in range(2):
                nc.tensor.matmul(wc_ps[:, mh: mh + 1],
                                 w_sb[0:1, mh * 128:(mh + 1) * 128],
                                 ones_sb[0:1, 0:1],
                                 start=True, stop=True, skip_group_check=True)
            for mh in range(2):
                nc.vector.tensor_copy(w_col[mh][:], wc_ps[:, mh: mh + 1])

            # ---- sim + t
            t_ps = m_psum.tile([1, 256], F32, tag="misc", name="t_ps")
            for mh in range(2):
                gs = GGqq_ps[mh][:, 0:256]
                num = small_pool.tile([128, 256], F32, tag="num")
                nc.vector.tensor_scalar_add(num[:], gs, SMOOTH)
                den = small_pool.tile([128, 256], F32, tag="den")
                nc.vector.tensor_sub(den[:], kk_bc[:], gs)
                nc.vector.tensor_scalar_add(den[:], den[:], qq_col[mh][:, 0:1])
                rec = small_pool.tile([128, 256], F32, tag="rec")
                nc.vector.reciprocal(rec[:], den[:])
                sim = small_pool.tile([128, 256], F32, tag="sim")
                nc.vector.tensor_mul(sim[:], num[:], rec[:])
                nc.tensor.matmul(t_ps[:], w_col[mh][:, 0:1], sim[:],
                                 start=(mh == 0), stop=(mh == 1),
                                 skip_group_check=True)
            t_row = small_pool.tile([1, 256], F32)
            nc.vector.tensor_scalar_add(t_row[:], t_ps[:], b_sb[0:1, 0:1])
            tb_ps = m_psum.tile([128, 256], F32, tag="misc", name="tb_ps")
            nc.tensor.matmul(tb_ps[:], ones_sb[0:1, :], t_row[0:1, :],
                             start=True, stop=True, skip_group_check=True)
            t_bc = small_pool.tile([128, 256], F32)
            nc.vector.tensor_copy(t_bc[:], tb_ps[:])

            if debug:
                gdbg = small_pool.tile([128, 512], F32, tag="gdbg")
                nc.vector.tensor_copy(gdbg[:], G_ps[:])
                nc.sync.dma_start(dbg_G[:, :], gdbg[:])
                qdbg = small_pool.tile([128, 2], F32, tag="qdbg")
                nc.vector.tensor_copy(qdbg[:, 0:1], qq_col[0][:])
                nc.vector.tensor_copy(qdbg[:, 1:2], qq_col[1][:])
                nc.sync.dma_start(dbg_qq[:, :], qdbg[:])
                kdbg = small_pool.tile([128, 2], F32, tag="kdbg")
                nc.vector.tensor_copy(kdbg[:, 0:1], kk_col[0][:])
                nc.vector.tensor_copy(kdbg[:, 1:2], kk_col[1][:])
                nc.sync.dma_start(dbg_kk[:, :], kdbg[:])
                nc.sync.dma_start(dbg_t[:, :], t_row[:])

            # ---- scale v and write out
            for ch in range(2):
                for i in range(8):
                    vt = v_pool.tile([128, 2048], F32)
                    nc.sync.dma_start(
                        vt[:].rearrange("p (hh w) -> p hh w", hh=16),
                        v[ch * 128:(ch + 1) * 128, i * 16:(i + 1) * 16, :],
                    )
                    sc = small_pool.tile([128, 128], F32, tag="scale")
                    for half in range(2):
                        off = (2 * ch + half) * 64 + i * 8
                        nc.vector.tensor_copy(
                            sc[half * 64:(half + 1) * 64, :].rearrange(
                                "p (j w) -> p j w", j=8
                            ),
                            t_bc[half * 64:(half + 1) * 64, off:off + 8]
                            .unsqueeze(2).broadcast_to((64, 8, 16)),
                        )
                    ot = o_pool.tile([128, 2048], F32)
                    nc.vector.tensor_mul(
                        ot[:].rearrange("p (hh w) -> p hh w", hh=16),
                        vt[:].rearrange("p (hh w) -> p hh w", hh=16),
                        sc[:].unsqueeze(1).broadcast_to((128, 16, 128)),
                    )
                    nc.sync.dma_start(
                        out[ch * 128:(ch + 1) * 128, i * 16:(i + 1) * 16, :],
                        ot[:].rearrange("p (hh w) -> p hh w", hh=16),
                    )

    split_excess_waits(nc)
    return nc


_NC_CACHE = None


def _aux_inputs():
    return {
        "ident": np.eye(128, dtype=np.float32),
        "ones_r": np.ones((1, 128), dtype=np.float32),
    }


def kernel(q, k, v, w_shrink, b_shrink):
    global _NC_CACHE
    if _NC_CACHE is None:
        _NC_CACHE = build_kernel()
    nc = _NC_CACHE
    aux = _aux_inputs()
    w2 = np.ascontiguousarray(w_shrink.reshape(1, 256).astype(np.float32))
    b2 = np.ascontiguousarray(b_shrink.reshape(1, 1).astype(np.float32))
    in_maps = []
    for c in range(N_CORES):
        m = {
            "q": np.ascontiguousarray(q[c], dtype=np.float32),
            "k": np.ascontiguousarray(k[c], dtype=np.float32),
            "v": np.ascontiguousarray(v[c], dtype=np.float32),
            "w_shrink": w2,
            "b_shrink": b2,
        }
        m.update(aux)
        in_maps.append(m)
    res = run_bass_kernel_spmd(nc, in_maps, core_ids=list(range(N_CORES)))
    return np.stack([res.results[c]["out"] for c in range(N_CORES)], axis=0)


# revision 17
# speedup vs baseline: 1.0155x; 1.0155x over previous
"""RelPatchAttention2D kernel for Trainium2 (Bass/Tile), data-parallel over
batch across 8 NeuronCores.

Per sample (one core): q,k,v [256,128,128] f32. Patches: (s0, i, j) with
s0 = C//64 (4), i = H//16 (8), j = W//16 (8) -> 256 patches of 64*16*16 =
16384 elements. Computes the all-pairs patch Gram matrix G = Qp @ Kp^T,
sim = (G+eps)/(qq[:,None]+kk[None,:]-G+eps), t = w @ sim + b, and
out = t[patch(v)] * v.

Pipeline per core:
  - q,k streamed in natural layout [C-half, hh-half band, W] tiles,
    reordered on DVE to (j, hh, ww)-blocked float32r layout, PE-transposed
    into patch-major W buffers (free = cc*256 + patch), phased over
    hh-halves so both W buffers fit in SBUF.
  - G (and the q/k self-Grams, for their diagonals qq/kk) accumulated in
    PSUM over 2 phases x 64 cc-chunks x 2 M-halves of [K=128, M=128,
    N=256] float32r matmuls.
  - diag extraction via identity-mask + row reduce; row<->column and
    row->all-partition broadcasts via tiny K=1 matmuls / PE transposes.
  - v scaled per patch with broadcast-AP DVE multiplies.
"""

import numpy as np

import concourse.bass as bass
import concourse.tile as tile
from concourse import mybir
from concourse.bass_utils import run_bass_kernel_spmd

F32 = mybir.dt.float32
F32R = mybir.dt.float32r
N_CORES = 8
C, H, W = 256, 128, 128
SMOOTH = 1e-05


def split_excess_waits(nc, max_waits=1):
    # This walrus build rejects >1 sync-wait per instruction ("Too many
    # sync wait commands"); move extras onto same-engine NOPs inserted
    # directly before the instruction.
    ctr = 0
    for f in nc.m.functions:
        for b in f.blocks:
            new_list = []
            changed = False
            for inst in b.instructions:
                si = getattr(inst, "sync_info", None)
                if si is not None and si.on_wait and len(si.on_wait) > max_waits:
                    waits = list(si.on_wait)
                    for w in waits[:-max_waits]:
                        nop = mybir.InstNoOp(name=f"wsplit-{ctr}", ins=[], outs=[])
                        ctr += 1
                        nop.engine = inst.engine
                        nop.sync_info = mybir.SyncInfo(on_wait=[w], on_update=[])
                        new_list.append(nop)
                    si.on_wait = waits[-max_waits:]
                    changed = True
                new_list.append(inst)
            if changed:
                b.instructions = new_list


def build_kernel(debug=False):
    nc = bass.Bass("TRN2", target_bir_lowering=False, debug=False)

    q = nc.dram_tensor("q", [C, H, W], F32, kind="ExternalInput").ap()
    k = nc.dram_tensor("k", [C, H, W], F32, kind="ExternalInput").ap()
    v = nc.dram_tensor("v", [C, H, W], F32, kind="ExternalInput").ap()
    w_shrink = nc.dram_tensor("w_shrink", [1, 256], F32, kind="ExternalInput").ap()
    b_shrink = nc.dram_tensor("b_shrink", [1, 1], F32, kind="ExternalInput").ap()
    ident = nc.dram_tensor("ident", [128, 128], F32, kind="ExternalInput").ap()
    ones_r = nc.dram_tensor("ones_r", [1, 128], F32, kind="ExternalInput").ap()
    out = nc.dram_tensor("out", [C, H, W], F32, kind="ExternalOutput").ap()
    if debug:
        dbg_G = nc.dram_tensor("dbg_G", [128, 512], F32, kind="ExternalOutput").ap()
        dbg_qq = nc.dram_tensor("dbg_qq", [128, 2], F32, kind="ExternalOutput").ap()
        dbg_kk = nc.dram_tensor("dbg_kk", [128, 2], F32, kind="ExternalOutput").ap()
        dbg_t = nc.dram_tensor("dbg_t", [1, 256], F32, kind="ExternalOutput").ap()

    qk_dram = (q, k)

    with tile.TileContext(nc) as tc:
        with (
            tc.tile_pool(name="aux", bufs=1) as aux_pool,
            tc.tile_pool(name="wbuf", bufs=1) as w_pool,
            tc.tile_pool(name="stage1", bufs=3) as s1_pool,
            tc.tile_pool(name="stage2", bufs=4) as s2_pool,
            tc.tile_pool(name="small", bufs=1) as small_pool,
            tc.tile_pool(name="vio", bufs=2) as v_pool,
            tc.tile_pool(name="voo", bufs=2) as o_pool,
            tc.tile_pool(name="tps", bufs=2, space="PSUM") as tp_psum,
            tc.tile_pool(name="gps", bufs=1, space="PSUM") as g_psum,
            tc.tile_pool(name="mps", bufs=1, space="PSUM") as m_psum,
        ):
            ident_sb = aux_pool.tile([128, 128], F32)
            nc.sync.dma_start(ident_sb[:], ident[:, :])
            identr_sb = aux_pool.tile([128, 128], F32R)
            nc.vector.tensor_copy(identr_sb[:], ident_sb[:])
            ones_sb = aux_pool.tile([1, 128], F32)
            nc.sync.dma_start(ones_sb[:], ones_r[:, :])
            w_sb = aux_pool.tile([1, 256], F32)
            nc.sync.dma_start(w_sb[:], w_shrink[:, :])
            b_sb = aux_pool.tile([1, 1], F32)
            nc.sync.dma_start(b_sb[:], b_shrink[:, :])

            # persistent merged W buffer: [128 hw, cc*512 + tk*256 + p]
            # float32r (tk: 0 = k, 1 = q), one hh-half phase at a time
            W_all = w_pool.tile([128, 64 * 512], F32R, name="W_all", tag="W_all")
            # Gram accumulators: GGqq[mh] = [G-row (vs k, 256) | Gqq-row
            # (vs q, 256)]; Gkk regions by M-half
            GGqq_ps = [
                g_psum.tile([128, 512], F32, name=f"GGqq{m}", tag=f"GGqq{m}")
                for m in range(2)
            ]
            Gkk_ps = g_psum.tile([128, 512], F32, name="Gkk_ps", tag="Gkk_ps")

            for h0 in range(2):
                for t in range(2):
                    src = qk_dram[t]
                    for ch in range(2):
                        for i in range(8):
                            st1 = s1_pool.tile([128, 1024], F32)
                            nc.sync.dma_start(
                                st1[:].rearrange("p (hh w) -> p hh w", hh=8),
                                src[
                                    ch * 128:(ch + 1) * 128,
                                    i * 16 + h0 * 8: i * 16 + h0 * 8 + 8,
                                    :,
                                ],
                            )
                            # reorder (hh, j, ww) -> (j, hh, ww), cast f32r
                            st2 = s2_pool.tile([128, 1024], F32R)
                            nc.vector.tensor_copy(
                                st2[:].rearrange("p (j hh w) -> p j hh w", j=8, hh=8),
                                st1[:].rearrange("p (hh j w) -> p j hh w", hh=8, j=8),
                            )
                            # transposes into W; 8 j-blocks -> one 2-bank
                            # PSUM group, one scattered copy (split ACT/DVE)
                            wv = W_all[:].rearrange(
                                "p (cc tk s i j) -> p tk j s cc i",
                                cc=64, tk=2, s=4, i=8, j=8,
                            )[:, 1 - t]
                            ps = tp_psum.tile([128, 1024], F32R)
                            for j in range(8):
                                nc.tensor.transpose(
                                    ps[:, j * 128:(j + 1) * 128],
                                    st2[:, j * 128:(j + 1) * 128],
                                    identr_sb[:],
                                )
                            wdst = wv[:, :, 2 * ch:2 * ch + 2, :, i]
                            wsrc = ps[:].rearrange(
                                "p (j s cc) -> p j s cc", j=8, s=2
                            )
                            nc.scalar.copy(wdst, wsrc)
                # Gram chunks for this phase: [G | Gqq] = Wq^T [Wk | Wq]
                # in one N=512 matmul per M-half; Gkk = Wk^T Wk (N=256)
                lw = W_all[:].rearrange("p (cc pp) -> p cc pp", cc=64)
                first = h0 == 0
                last = h0 == 1
                for cc_i in range(64):
                    # start=True clears the WHOLE PSUM bank, so only the
                    # very first matmul into each bank may carry it; other
                    # groups in the bank begin with start=False (their
                    # region's has_written is clear, so the first write
                    # lands in overwrite mode).
                    st = (first and cc_i == 0)
                    sp = (last and cc_i == 63)
                    for mh in range(2):
                        lhs_q = lw[:, cc_i, 256 + mh * 128: 256 + (mh + 1) * 128]
                        lhs_k = lw[:, cc_i, mh * 128:(mh + 1) * 128]
                        nc.tensor.matmul(
                            GGqq_ps[mh][:],
                            lhs_q, lw[:, cc_i, :],
                            start=st, stop=sp,
                            skip_group_check=True,
                        )
                        nc.tensor.matmul(
                            Gkk_ps[:, mh * 256:(mh + 1) * 256],
                            lhs_k, lw[:, cc_i, 0:256],
                            start=(st and mh == 0), stop=sp,
                            skip_group_check=True,
                        )

            # ---- qq/kk columns: diag(selfgram) via identity mask + reduce
            qq_col = [
                small_pool.tile([128, 1], F32, name=f"qqc{m}", tag=f"qqc{m}")
                for m in range(2)
            ]
            kk_col = [
                small_pool.tile([128, 1], F32, name=f"kkc{m}", tag=f"kkc{m}")
                for m in range(2)
            ]
            w_col = [
                small_pool.tile([128, 1], F32, name=f"wc{m}", tag=f"wc{m}")
                for m in range(2)
            ]
            dtmp = small_pool.tile([128, 128], F32, name="dtmp", tag="dtmp")
            for mh in range(2):
                for gslice, col in (
                    (GGqq_ps[mh][:, 256 + mh * 128: 256 + (mh + 1) * 128],
                     qq_col[mh]),
                    (Gkk_ps[:, mh * 256 + mh * 128: mh * 256 + (mh + 1) * 128],
                     kk_col[mh]),
                ):
                    nc.vector.tensor_mul(dtmp[:], gslice, ident_sb[:])
                    nc.vector.tensor_reduce(
                        col[:], dtmp[:],
                        axis=mybir.AxisListType.X, op=mybir.AluOpType.add,
                    )
            # eps folded into qq
            for mh in range(2):
                nc.vector.tensor_scalar_add(qq_col[mh][:], qq_col[mh][:], SMOOTH)

            # kk as a broadcast row: transpose columns -> row, then K=1 bcast
            kr_ps = m_psum.tile([1, 256], F32, tag="misc", name="kr_ps")
            for mh in range(2):
                nc.tensor.transpose(
                    kr_ps[0:1, mh * 128:(mh + 1) * 128], kk_col[mh][:, 0:1],
                    ident_sb[:],
                )
            kk_row = small_pool.tile([1, 256], F32)
            nc.vector.tensor_copy(kk_row[:], kr_ps[:])
            kkb_ps = m_psum.tile([128, 256], F32, tag="misc", name="kkb_ps")
            nc.tensor.matmul(kkb_ps[:], ones_sb[0:1, :], kk_row[0:1, :],
                             start=True, stop=True, skip_group_check=True)
            kk_bc = small_pool.tile([128, 256], F32)
            nc.vector.tensor_copy(kk_bc[:], kkb_ps[:])

            # w as per-partition columns
            wc_ps = m_psum.tile([128, 64], F32, tag="misc", name="wc_ps")
            for mh in range(2):
                nc.tensor.matmul(wc_ps[:, mh: mh + 1],
                                 w_sb[0:1, mh * 128:(mh + 1) * 128],
                                 ones_sb[0:1, 0:1],
                                 start=True, stop=True, skip_group_check=True)
            for mh in range(2):
                nc.vector.tensor_copy(w_col[mh][:], wc_ps[:, mh: mh + 1])

            # ---- sim + t
            t_ps = m_psum.tile([1, 256], F32, tag="misc", name="t_ps")
            for mh in range(2):
                gs = GGqq_ps[mh][:, 0:256]
                num = small_pool.tile([128, 256], F32, tag="num")
                nc.vector.tensor_scalar_add(num[:], gs, SMOOTH)
                den = small_pool.tile([128, 256], F32, tag="den")
                nc.vector.tensor_sub(den[:], kk_bc[:], gs)
                nc.vector.tensor_scalar_add(den[:], den[:], qq_col[mh][:, 0:1])
                rec = small_pool.tile([128, 256], F32, tag="rec")
                nc.vector.reciprocal(rec[:], den[:])
                sim = small_pool.tile([128, 256], F32, tag="sim")
                nc.vector.tensor_mul(sim[:], num[:], rec[:])
                nc.tensor.matmul(t_ps[:], w_col[mh][:, 0:1], sim[:],
                                 start=(mh == 0), stop=(mh == 1),
                                 skip_group_check=True)
            t_row = small_pool.tile([1, 256], F32)
            nc.vector.tensor_scalar_add(t_row[:], t_ps[:], b_sb[0:1, 0:1])
            tb_ps = m_psum.tile([128, 256], F32, tag="misc", name="tb_ps")
            nc.tensor.matmul(tb_ps[:], ones_sb[0:1, :], t_row[0:1, :],
                             start=True, stop=True, skip_group_check=True)
            t_bc = small_pool.tile([128, 256], F32)
            nc.vector.tensor_copy(t_bc[:], tb_ps[:])

            if debug:
                gdbg = small_pool.tile([128, 512], F32, tag="gdbg")
                nc.vector.tensor_copy(gdbg[:], G_ps[:])
                nc.sync.dma_start(dbg_G[:, :], gdbg[:])
                qdbg = small_pool.tile([128, 2], F32, tag="qdbg")
                nc.vector.tensor_copy(qdbg[:, 0:1], qq_col[0][:])
                nc.vector.tensor_copy(qdbg[:, 1:2], qq_col[1][:])
                nc.sync.dma_start(dbg_qq[:, :], qdbg[:])
                kdbg = small_pool.tile([128, 2], F32, tag="kdbg")
                nc.vector.tensor_copy(kdbg[:, 0:1], kk_col[0][:])
                nc.vector.tensor_copy(kdbg[:, 1:2], kk_col[1][:])
                nc.sync.dma_start(dbg_kk[:, :], kdbg[:])
                nc.sync.dma_start(dbg_t[:, :], t_row[:])

            # ---- scale v and write out
            for ch in range(2):
                for i in range(8):
                    vt = v_pool.tile([128, 2048], F32)
                    nc.sync.dma_start(
                        vt[:].rearrange("p (hh w) -> p hh w", hh=16),
                        v[ch * 128:(ch + 1) * 128, i * 16:(i + 1) * 16, :],
                    )
                    sc = small_pool.tile([128, 128], F32, tag="scale")
                    for half in range(2):
                        off = (2 * ch + half) * 64 + i * 8
                        nc.vector.tensor_copy(
                            sc[half * 64:(half + 1) * 64, :].rearrange(
                                "p (j w) -> p j w", j=8
                            ),
                            t_bc[half * 64:(half + 1) * 64, off:off + 8]
                            .unsqueeze(2).broadcast_to((64, 8, 16)),
                        )
                    ot = o_pool.tile([128, 2048], F32)
                    nc.vector.tensor_mul(
                        ot[:].rearrange("p (hh w) -> p hh w", hh=16),
                        vt[:].rearrange("p (hh w) -> p hh w", hh=16),
                        sc[:].unsqueeze(1).broadcast_to((128, 16, 128)),
                    )
                    nc.sync.dma_start(
                        out[ch * 128:(ch + 1) * 128, i * 16:(i + 1) * 16, :],
                        ot[:].rearrange("p (hh w) -> p hh w", hh=16),
                    )

    split_excess_waits(nc)
    return nc


_NC_CACHE = None


def _aux_inputs():
    return {
        "ident": np.eye(128, dtype=np.float32),
        "ones_r": np.ones((1, 128), dtype=np.float32),
    }


def kernel(q, k, v, w_shrink, b_shrink):
    global _NC_CACHE
    if _NC_CACHE is None:
        _NC_CACHE = build_kernel()
    nc = _NC_CACHE
    aux = _aux_inputs()
    w2 = np.ascontiguousarray(w_shrink.reshape(1, 256).astype(np.float32))
    b2 = np.ascontiguousarray(b_shrink.reshape(1, 1).astype(np.float32))
    in_maps = []
    for c in range(N_CORES):
        m = {
            "q": np.ascontiguousarray(q[c], dtype=np.float32),
            "k": np.ascontiguousarray(k[c], dtype=np.float32),
            "v": np.ascontiguousarray(v[c], dtype=np.float32),
            "w_shrink": w2,
            "b_shrink": b2,
        }
        m.update(aux)
        in_maps.append(m)
    res = run_bass_kernel_spmd(nc, in_maps, core_ids=list(range(N_CORES)))
    return np.stack([res.results[c]["out"] for c in range(N_CORES)], axis=0)


# revision 18
# speedup vs baseline: 1.1399x; 1.1224x over previous
"""RelPatchAttention2D kernel for Trainium2 (Bass/Tile), data-parallel over
batch across 8 NeuronCores.

Per sample (one core): q,k,v [256,128,128] f32. Patches: (s0, i, j) with
s0 = C//64 (4), i = H//16 (8), j = W//16 (8) -> 256 patches of 64*16*16 =
16384 elements. Computes the all-pairs patch Gram matrix G = Qp @ Kp^T,
sim = (G+eps)/(qq[:,None]+kk[None,:]-G+eps), t = w @ sim + b, and
out = t[patch(v)] * v.

Pipeline per core:
  - q,k streamed in natural layout [C-half, hh-half band, W] tiles,
    reordered on DVE to (j, hh, ww)-blocked float32r layout, PE-transposed
    into patch-major W buffers (free = cc*256 + patch), phased over
    hh-halves so both W buffers fit in SBUF.
  - G (and the q/k self-Grams, for their diagonals qq/kk) accumulated in
    PSUM over 2 phases x 64 cc-chunks x 2 M-halves of [K=128, M=128,
    N=256] float32r matmuls.
  - diag extraction via identity-mask + row reduce; row<->column and
    row->all-partition broadcasts via tiny K=1 matmuls / PE transposes.
  - v scaled per patch with broadcast-AP DVE multiplies.
"""

import numpy as np

import concourse.bass as bass
import concourse.tile as tile
from concourse import mybir
from concourse.bass_utils import run_bass_kernel_spmd

F32 = mybir.dt.float32
F32R = mybir.dt.float32r
N_CORES = 8
C, H, W = 256, 128, 128
SMOOTH = 1e-05


def split_excess_waits(nc, max_waits=1):
    # This walrus build rejects >1 sync-wait per instruction ("Too many
    # sync wait commands"); move extras onto same-engine NOPs inserted
    # directly before the instruction.
    ctr = 0
    for f in nc.m.functions:
        for b in f.blocks:
            new_list = []
            changed = False
            for inst in b.instructions:
                si = getattr(inst, "sync_info", None)
                if si is not None and si.on_wait and len(si.on_wait) > max_waits:
                    waits = list(si.on_wait)
                    for w in waits[:-max_waits]:
                        nop = mybir.InstNoOp(name=f"wsplit-{ctr}", ins=[], outs=[])
                        ctr += 1
                        nop.engine = inst.engine
                        nop.sync_info = mybir.SyncInfo(on_wait=[w], on_update=[])
                        new_list.append(nop)
                    si.on_wait = waits[-max_waits:]
                    changed = True
                new_list.append(inst)
            if changed:
                b.instructions = new_list


def build_kernel(debug=False):
    nc = bass.Bass("TRN2", target_bir_lowering=False, debug=False)

    q = nc.dram_tensor("q", [C, H, W], F32, kind="ExternalInput").ap()
    k = nc.dram_tensor("k", [C, H, W], F32, kind="ExternalInput").ap()
    v = nc.dram_tensor("v", [C, H, W], F32, kind="ExternalInput").ap()
    w_shrink = nc.dram_tensor("w_shrink", [1, 256], F32, kind="ExternalInput").ap()
    b_shrink = nc.dram_tensor("b_shrink", [1, 1], F32, kind="ExternalInput").ap()
    ident = nc.dram_tensor("ident", [128, 128], F32, kind="ExternalInput").ap()
    ones_r = nc.dram_tensor("ones_r", [1, 128], F32, kind="ExternalInput").ap()
    out = nc.dram_tensor("out", [C, H, W], F32, kind="ExternalOutput").ap()
    if debug:
        dbg_G = nc.dram_tensor("dbg_G", [128, 512], F32, kind="ExternalOutput").ap()
        dbg_qq = nc.dram_tensor("dbg_qq", [128, 2], F32, kind="ExternalOutput").ap()
        dbg_kk = nc.dram_tensor("dbg_kk", [128, 2], F32, kind="ExternalOutput").ap()
        dbg_t = nc.dram_tensor("dbg_t", [1, 256], F32, kind="ExternalOutput").ap()

    qk_dram = (q, k)

    with tile.TileContext(nc) as tc:
        with (
            tc.tile_pool(name="aux", bufs=1) as aux_pool,
            tc.tile_pool(name="wbuf", bufs=1) as w_pool,
            tc.tile_pool(name="stage1", bufs=3) as s1_pool,
            tc.tile_pool(name="stage2", bufs=3) as s2_pool,
            tc.tile_pool(name="small", bufs=1) as small_pool,
            tc.tile_pool(name="vio", bufs=2) as v_pool,
            tc.tile_pool(name="voo", bufs=2) as o_pool,
            tc.tile_pool(name="tps", bufs=2, space="PSUM") as tp_psum,
            tc.tile_pool(name="gps", bufs=1, space="PSUM") as g_psum,
            tc.tile_pool(name="mps", bufs=1, space="PSUM") as m_psum,
        ):
            ident_sb = aux_pool.tile([128, 128], F32)
            nc.sync.dma_start(ident_sb[:], ident[:, :])
            identr_sb = aux_pool.tile([128, 128], F32R)
            nc.vector.tensor_copy(identr_sb[:], ident_sb[:])
            ones_sb = aux_pool.tile([1, 128], F32)
            nc.sync.dma_start(ones_sb[:], ones_r[:, :])
            w_sb = aux_pool.tile([1, 256], F32)
            nc.sync.dma_start(w_sb[:], w_shrink[:, :])
            b_sb = aux_pool.tile([1, 1], F32)
            nc.sync.dma_start(b_sb[:], b_shrink[:, :])

            # persistent W buffers: [128 hw, cc*256 + p] float32r, one
            # hh-half phase at a time, q and k
            Wb = [
                w_pool.tile([128, 64 * 256], F32R, name=f"Wb{t}", tag=f"Wb{t}")
                for t in range(2)
            ]
            # Gram accumulators: [:, 0:256] = M-half 0, [:, 256:512] = half 1
            G_ps = g_psum.tile([128, 512], F32, name="G_ps", tag="G_ps")
            Gqq_ps = g_psum.tile([128, 512], F32, name="Gqq_ps", tag="Gqq_ps")
            Gkk_ps = g_psum.tile([128, 512], F32, name="Gkk_ps", tag="Gkk_ps")

            for h0 in range(2):
                for t in range(2):
                    src = qk_dram[t]
                    for ch in range(2):
                        for i in range(8):
                            st1 = s1_pool.tile([128, 1024], F32)
                            nc.sync.dma_start(
                                st1[:].rearrange("p (hh w) -> p hh w", hh=8),
                                src[
                                    ch * 128:(ch + 1) * 128,
                                    i * 16 + h0 * 8: i * 16 + h0 * 8 + 8,
                                    :,
                                ],
                            )
                            # reorder (hh, j, ww) -> (j, hh, ww), cast f32r
                            st2 = s2_pool.tile([128, 1024], F32R)
                            nc.vector.tensor_copy(
                                st2[:].rearrange("p (j hh w) -> p j hh w", j=8, hh=8),
                                st1[:].rearrange("p (hh j w) -> p j hh w", hh=8, j=8),
                            )
                            # transposes into W; 8 j-blocks -> one 2-bank
                            # PSUM group, one scattered copy (split ACT/DVE)
                            wv = Wb[t][:].rearrange(
                                "p (cc s i j) -> p j s cc i", cc=64, s=4, i=8, j=8
                            )
                            ps = tp_psum.tile([128, 1024], F32R)
                            for j in range(8):
                                nc.tensor.transpose(
                                    ps[:, j * 128:(j + 1) * 128],
                                    st2[:, j * 128:(j + 1) * 128],
                                    identr_sb[:],
                                )
                            wdst = wv[:, :, 2 * ch:2 * ch + 2, :, i]
                            wsrc = ps[:].rearrange(
                                "p (j s cc) -> p j s cc", j=8, s=2
                            )
                            nc.scalar.copy(wdst, wsrc)
                # Gram chunks for this phase: G = Wq^T Wk, plus self-Grams
                # for the qq/kk diagonals
                lq = Wb[0][:].rearrange("p (cc pp) -> p cc pp", cc=64)
                lk = Wb[1][:].rearrange("p (cc pp) -> p cc pp", cc=64)
                first = h0 == 0
                last = h0 == 1
                for cc_i in range(64):
                    # start=True clears the WHOLE PSUM bank, so only the
                    # very first matmul into each bank may carry it; other
                    # groups in the bank begin with start=False (their
                    # region's has_written is clear, so the first write
                    # lands in overwrite mode).
                    st = (first and cc_i == 0)
                    sp = (last and cc_i == 63)
                    for mh in range(2):
                        lhs_q = lq[:, cc_i, mh * 128:(mh + 1) * 128]
                        lhs_k = lk[:, cc_i, mh * 128:(mh + 1) * 128]
                        nc.tensor.matmul(
                            G_ps[:, mh * 256:(mh + 1) * 256],
                            lhs_q, lk[:, cc_i, :],
                            start=(st and mh == 0), stop=sp,
                            skip_group_check=True,
                        )
                        nc.tensor.matmul(
                            Gqq_ps[:, mh * 256:(mh + 1) * 256],
                            lhs_q, lq[:, cc_i, :],
                            start=(st and mh == 0), stop=sp,
                            skip_group_check=True,
                        )
                        nc.tensor.matmul(
                            Gkk_ps[:, mh * 256:(mh + 1) * 256],
                            lhs_k, lk[:, cc_i, :],
                            start=(st and mh == 0), stop=sp,
                            skip_group_check=True,
                        )

            # ---- qq/kk columns: diag(selfgram) via identity mask + reduce
            qq_col = [
                small_pool.tile([128, 1], F32, name=f"qqc{m}", tag=f"qqc{m}")
                for m in range(2)
            ]
            kk_col = [
                small_pool.tile([128, 1], F32, name=f"kkc{m}", tag=f"kkc{m}")
                for m in range(2)
            ]
            w_col = [
                small_pool.tile([128, 1], F32, name=f"wc{m}", tag=f"wc{m}")
                for m in range(2)
            ]
            dtmp = small_pool.tile([128, 128], F32, name="dtmp", tag="dtmp")
            for mh in range(2):
                for gsrc, col in ((Gqq_ps, qq_col[mh]), (Gkk_ps, kk_col[mh])):
                    nc.vector.tensor_mul(
                        dtmp[:],
                        gsrc[:, mh * 256 + mh * 128: mh * 256 + (mh + 1) * 128],
                        ident_sb[:],
                    )
                    nc.vector.tensor_reduce(
                        col[:], dtmp[:],
                        axis=mybir.AxisListType.X, op=mybir.AluOpType.add,
                    )
            # eps folded into qq
            for mh in range(2):
                nc.vector.tensor_scalar_add(qq_col[mh][:], qq_col[mh][:], SMOOTH)

            # kk as a broadcast row: transpose columns -> row, then K=1 bcast
            kr_ps = m_psum.tile([1, 256], F32, tag="misc", name="kr_ps")
            for mh in range(2):
                nc.tensor.transpose(
                    kr_ps[0:1, mh * 128:(mh + 1) * 128], kk_col[mh][:, 0:1],
                    ident_sb[:],
                )
            kk_row = small_pool.tile([1, 256], F32)
            nc.vector.tensor_copy(kk_row[:], kr_ps[:])
            kkb_ps = m_psum.tile([128, 256], F32, tag="misc", name="kkb_ps")
            nc.tensor.matmul(kkb_ps[:], ones_sb[0:1, :], kk_row[0:1, :],
                             start=True, stop=True, skip_group_check=True)
            kk_bc = small_pool.tile([128, 256], F32)
            nc.vector.tensor_copy(kk_bc[:], kkb_ps[:])

            # w as per-partition columns
            wc_ps = m_psum.tile([128, 64], F32, tag="misc", name="wc_ps")
            for mh in range(2):
                nc.tensor.matmul(wc_ps[:, mh: mh + 1],
                                 w_sb[0:1, mh * 128:(mh + 1) * 128],
                                 ones_sb[0:1, 0:1],
                                 start=True, stop=True, skip_group_check=True)
            for mh in range(2):
                nc.vector.tensor_copy(w_col[mh][:], wc_ps[:, mh: mh + 1])

            # ---- sim + t
            t_ps = m_psum.tile([1, 256], F32, tag="misc", name="t_ps")
            for mh in range(2):
                gs = G_ps[:, mh * 256:(mh + 1) * 256]
                num = small_pool.tile([128, 256], F32, tag="num")
                nc.vector.tensor_scalar_add(num[:], gs, SMOOTH)
                den = small_pool.tile([128, 256], F32, tag="den")
                nc.vector.tensor_sub(den[:], kk_bc[:], gs)
                nc.vector.tensor_scalar_add(den[:], den[:], qq_col[mh][:, 0:1])
                rec = small_pool.tile([128, 256], F32, tag="rec")
                nc.vector.reciprocal(rec[:], den[:])
                sim = small_pool.tile([128, 256], F32, tag="sim")
                nc.vector.tensor_mul(sim[:], num[:], rec[:])
                nc.tensor.matmul(t_ps[:], w_col[mh][:, 0:1], sim[:],
                                 start=(mh == 0), stop=(mh == 1),
                                 skip_group_check=True)
            t_row = small_pool.tile([1, 256], F32)
            nc.vector.tensor_scalar_add(t_row[:], t_ps[:], b_sb[0:1, 0:1])
            tb_ps = m_psum.tile([128, 256], F32, tag="misc", name="tb_ps")
            nc.tensor.matmul(tb_ps[:], ones_sb[0:1, :], t_row[0:1, :],
                             start=True, stop=True, skip_group_check=True)
            t_bc = small_pool.tile([128, 256], F32)
            nc.vector.tensor_copy(t_bc[:], tb_ps[:])

            if debug:
                gdbg = small_pool.tile([128, 512], F32, tag="gdbg")
                nc.vector.tensor_copy(gdbg[:], G_ps[:])
                nc.sync.dma_start(dbg_G[:, :], gdbg[:])
                qdbg = small_pool.tile([128, 2], F32, tag="qdbg")
                nc.vector.tensor_copy(qdbg[:, 0:1], qq_col[0][:])
                nc.vector.tensor_copy(qdbg[:, 1:2], qq_col[1][:])
                nc.sync.dma_start(dbg_qq[:, :], qdbg[:])
                kdbg = small_pool.tile([128, 2], F32, tag="kdbg")
                nc.vector.tensor_copy(kdbg[:, 0:1], kk_col[0][:])
                nc.vector.tensor_copy(kdbg[:, 1:2], kk_col[1][:])
                nc.sync.dma_start(dbg_kk[:, :], kdbg[:])
                nc.sync.dma_start(dbg_t[:, :], t_row[:])

            # ---- scale v and write out
            for ch in range(2):
                for i in range(8):
                    vt = v_pool.tile([128, 2048], F32)
                    nc.sync.dma_start(
                        vt[:].rearrange("p (hh w) -> p hh w", hh=16),
                        v[ch * 128:(ch + 1) * 128, i * 16:(i + 1) * 16, :],
                    )
                    sc = small_pool.tile([128, 128], F32, tag="scale")
                    for half in range(2):
                        off = (2 * ch + half) * 64 + i * 8
                        nc.vector.tensor_copy(
                            sc[half * 64:(half + 1) * 64, :].rearrange(
                                "p (j w) -> p j w", j=8
                            ),
                            t_bc[half * 64:(half + 1) * 64, off:off + 8]
                            .unsqueeze(2).broadcast_to((64, 8, 16)),
                        )
                    ot = o_pool.tile([128, 2048], F32)
                    nc.vector.tensor_mul(
                        ot[:].rearrange("p (hh w) -> p hh w", hh=16),
                        vt[:].rearrange("p (hh w) -> p hh w", hh=16),
                        sc[:].unsqueeze(1).broadcast_to((128, 16, 128)),
                    )
                    nc.sync.dma_start(
                        out[ch * 128:(ch + 1) * 128, i * 16:(i + 1) * 16, :],
                        ot[:].rearrange("p (hh w) -> p hh w", hh=16),
                    )

    split_excess_waits(nc)
    return nc


_NC_CACHE = None


def _aux_inputs():
    return {
        "ident": np.eye(128, dtype=np.float32),
        "ones_r": np.ones((1, 128), dtype=np.float32),
    }


def kernel(q, k, v, w_shrink, b_shrink):
    global _NC_CACHE
    if _NC_CACHE is None:
        _NC_CACHE = build_kernel()
    nc = _NC_CACHE
    aux = _aux_inputs()
    w2 = np.ascontiguousarray(w_shrink.reshape(1, 256).astype(np.float32))
    b2 = np.ascontiguousarray(b_shrink.reshape(1, 1).astype(np.float32))
    in_maps = []
    for c in range(N_CORES):
        m = {
            "q": np.ascontiguousarray(q[c], dtype=np.float32),
            "k": np.ascontiguousarray(k[c], dtype=np.float32),
            "v": np.ascontiguousarray(v[c], dtype=np.float32),
            "w_shrink": w2,
            "b_shrink": b2,
        }
        m.update(aux)
        in_maps.append(m)
    res = run_bass_kernel_spmd(nc, in_maps, core_ids=list(range(N_CORES)))
    return np.stack([res.results[c]["out"] for c in range(N_CORES)], axis=0)
